# revision 22
# baseline (speedup 1.0000x reference)
"""Multi-head self-attention Trainium2 Bass kernel.

Full-input contract: kernel(**inputs) takes the unsharded inputs
(x [4,2048,1024], Wq [1024,512], bq [512], Wk, bk, Wv [1024,1024], bv)
and returns the full [4,2048,1024] output.

Sharding: 8 cores = 4 batches x 2 head-groups. Core c handles batch c//2
and heads 4*(c%2) .. 4*(c%2)+4. Pure SPMD, no collectives.

Per-core algorithm (N=2048 rows, C=1024, 4 heads, d=64, v=128):
  - load x naturally, PE-transpose into xT (C on partitions)
  - QT/KT = W.T @ xT with head-dim on partitions; V natural (rows on
    partitions); biases folded in via rank-1 (K=1) matmuls
  - scores computed TRANSPOSED: sT[keys,q] = (KT tile).T @ QT, so that
    exp(sT) (ACT, scale fused) is directly the PV rhs operand. No max
    subtraction (|scale*s| < ~4 -> exp safely in fp32 range).
  - msgT[v,q] accumulates over key tiles; row-sums of exp via ones-lhsT
    matmuls (column-tiled 4-wide so 4 run concurrently on the PE array);
    per-q normalization applied after transposing back to natural layout.
"""

import math
import os

import numpy as np

import concourse.bass as bass
import concourse.mybir as mybir
import concourse.tile as tile
from concourse import bacc
from concourse.bass_utils import run_bass_kernel_spmd
from concourse.masks import make_identity

F32 = mybir.dt.float32
BF16 = mybir.dt.bfloat16
F32R = mybir.dt.float32r

# dims
B, N, C = 4, 2048, 1024
QK_DIM, NHEADS = 512, 8
D = QK_DIM // NHEADS          # 64 per-head qk dim
V = 1024 // NHEADS            # 128 per-head value dim
SCALE = 1.0 / math.sqrt(D)
HC = 4                        # heads per core
P = 128
NT = N // P                   # 16 row tiles
CT = C // P                   # 8 contraction tiles
KT = N // P                   # 16 key tiles
QC = 4                        # q chunks of 512
QW = N // QC                  # 512


def build_nc(mode: str = "bf16", repeat: int = 1):
    """Build the per-core Bass program (bf16 matmul operands, fp32 psum)."""
    mmdt = BF16
    mmcast = lambda ap: ap

    nc = bacc.Bacc("TRN2", target_bir_lowering=False, debug=False, num_devices=8)

    x_d = nc.dram_tensor("x", [N, C], F32, kind="ExternalInput").ap()
    wq_d = nc.dram_tensor("wq", [C, HC * D], F32, kind="ExternalInput").ap()
    bq_d = nc.dram_tensor("bq", [HC * D], F32, kind="ExternalInput").ap()
    wk_d = nc.dram_tensor("wk", [C, HC * D], F32, kind="ExternalInput").ap()
    bk_d = nc.dram_tensor("bk", [HC * D], F32, kind="ExternalInput").ap()
    wv_d = nc.dram_tensor("wv", [C, HC * V], F32, kind="ExternalInput").ap()
    bv_d = nc.dram_tensor("bv", [HC * V], F32, kind="ExternalInput").ap()
    out_d = nc.dram_tensor("out", [N, HC * V], F32, kind="ExternalOutput").ap()

    with tile.TileContext(nc) as tc:
      for _rep in range(repeat):
        with tc.tile_pool(name="persist", bufs=1) as persist:
            # persistent SBUF arrays
            xT = [persist.tile([P, N], mmdt, tag=f"xT{ct}", name=f"xT{ct}") for ct in range(CT)]
            QT = [persist.tile([P, N], mmdt, tag=f"QT{hp}", name=f"QT{hp}") for hp in range(2)]
            KTt = [persist.tile([P, N], mmdt, tag=f"KT{hp}", name=f"KT{hp}") for hp in range(2)]
            Vt = [persist.tile([P, HC * V], mmdt, tag=f"V{rt}", name=f"V{rt}") for rt in range(NT)]

            ident = persist.tile([P, P], mmdt, tag="ident")
            make_identity(nc, ident)
            identf = persist.tile([P, P], F32, tag="identf")
            make_identity(nc, identf)

            ones_row = persist.tile([1, QW], mmdt, tag="ones_row")
            nc.vector.memset(ones_row[:], 1.0)
            ones32 = persist.tile([P, 32], mmdt, tag="ones32")
            nc.vector.memset(ones32[:], 1.0)
            inv32 = persist.tile([P, 1], F32, tag="inv32")
            nc.vector.memset(inv32[:], 1.0 / 32.0)

            # biases (as [1, n] rows in matmul dtype)
            bq_sb = persist.tile([1, HC * D], mmdt, tag="bq")
            bk_sb = persist.tile([1, HC * D], mmdt, tag="bk")
            bv_sb = persist.tile([1, HC * V], mmdt, tag="bv")
            # weights in matmul dtype
            wv_sb = [persist.tile([P, HC * V], mmdt, tag=f"wv{ct}", name=f"wv{ct}") for ct in range(CT)]
            wq_sb = [persist.tile([P, HC * D], mmdt, tag=f"wqf{ct}", name=f"wqf{ct}") for ct in range(CT)]
            wk_sb = [persist.tile([P, HC * D], mmdt, tag=f"wkf{ct}", name=f"wkf{ct}") for ct in range(CT)]

            # ---- Phase A: load x (critical path: issue x DMAs first),
            #      transpose into xT ----
            with tc.tile_pool(name="xload", bufs=3) as xload, \
                 tc.tile_pool(name="stage", bufs=3) as stage, \
                 tc.tile_pool(name="tp_psum", bufs=3, space="PSUM") as tp_psum, \
                 tc.tile_pool(name="pp_psum", bufs=3, space="PSUM") as pp_psum:
                xns = []
                for rt in range(NT):
                    xn = xload.tile([P, C], F32, tag="xn", bufs=6, name="xn")
                    eng = nc.sync if rt % 2 == 0 else nc.scalar
                    eng.dma_start(out=xn[:], in_=x_d[rt * P:(rt + 1) * P, :])
                    xns.append(xn)

                # weight/bias staging on a different DMA engine so it does
                # not delay the x loads
                for bd, bs, n_ in ((bq_d, bq_sb, HC * D), (bk_d, bk_sb, HC * D),
                                   (bv_d, bv_sb, HC * V)):
                    st = stage.tile([1, n_], F32, tag="bias_st", name="b_st")
                    nc.scalar.dma_start(out=st[:], in_=bd.unsqueeze(0))
                    nc.vector.tensor_copy(bs[:], st[:])
                for ct in range(CT):
                    for wd, ws, n_ in ((wq_d, wq_sb[ct], HC * D),
                                       (wk_d, wk_sb[ct], HC * D),
                                       (wv_d, wv_sb[ct], HC * V)):
                        st = stage.tile([P, n_], F32, tag="w_st", name="w_st")
                        nc.scalar.dma_start(out=st[:], in_=wd[ct * P:(ct + 1) * P, :])
                        nc.vector.tensor_copy(ws[:], st[:])

                for rt in range(NT):
                    xn = xns[rt]
                    xb = xload.tile([P, C], mmdt, tag="xb", name="xb")
                    nc.vector.tensor_copy(xb[:], xn[:])
                    for ct in range(CT):
                        pt = tp_psum.tile([P, P], mmdt, name="pt")
                        nc.tensor.transpose(pt[:], xb[:, ct * P:(ct + 1) * P], ident[:])
                        # alternate eviction engine to split the load
                        if ct % 2 == 0:
                            nc.vector.tensor_copy(xT[ct][:, rt * P:(rt + 1) * P], pt[:])
                        else:
                            nc.scalar.copy(xT[ct][:, rt * P:(rt + 1) * P], pt[:])

                # ---- Phases B+C interleaved ----
            # Phase C is ACT(exp)-bound while projections are PE-only, so
            # projection blocks are emitted INTO the attention loop where the
            # PE would otherwise idle. One 2-slot PSUM scratch ring serves
            # proj blocks, sums banks and output transposes (their lifetimes
            # never overlap).
            n_pt_bufs = KT + 2
            with tc.tile_pool(name="sT_psum", bufs=2, space="PSUM") as sT_psum, \
                 tc.tile_pool(name="mT_psum", bufs=2, space="PSUM") as mT_psum, \
                 tc.tile_pool(name="scr_psum", bufs=2, space="PSUM") as scr_psum, \
                 tc.tile_pool(name="pT_pool", bufs=n_pt_bufs) as pT_pool, \
                 tc.tile_pool(name="cwork", bufs=2) as cwork:

                def emit_qk_block(hp, qc, which):
                    w_sb, b_sb, dst = ((wq_sb, bq_sb, QT) if which == 0
                                       else (wk_sb, bk_sb, KTt))
                    ps = scr_psum.tile([P, QW], F32, tag="scr", name="ppqk")
                    for ct in range(CT):
                        nc.tensor.matmul(
                            ps[:],
                            mmcast(w_sb[ct][:, hp * P:(hp + 1) * P]),
                            mmcast(xT[ct][:, qc * QW:(qc + 1) * QW]),
                            start=(ct == 0), stop=False)
                    nc.tensor.matmul(
                        ps[:], mmcast(b_sb[:, hp * P:(hp + 1) * P]),
                        mmcast(ones_row[:]), start=False, stop=True)
                    nc.vector.tensor_copy(dst[hp][:, qc * QW:(qc + 1) * QW], ps[:])

                def emit_v_block(rt):
                    ps = scr_psum.tile([P, HC * V], F32, tag="scr", name="ppv")
                    for ct in range(CT):
                        nc.tensor.matmul(
                            ps[:],
                            mmcast(xT[ct][:, rt * P:(rt + 1) * P]),
                            mmcast(wv_sb[ct][:]),
                            start=(ct == 0), stop=False)
                    nc.tensor.matmul(
                        ps[:], mmcast(ones_row[:, 0:P]),
                        mmcast(bv_sb[:]), start=False, stop=True)
                    nc.vector.tensor_copy(Vt[rt][:], ps[:])

                units = [(hp, qc) for hp in range(2) for qc in range(QC)]
                # pending projection work, consumed during attention loops.
                # Deps: unit (hp,qc) needs Q(hp,qc) at start, K(hp,c) by
                # group 2c (its sT scans ALL key chunks), V[k] by the group
                # whose (pipelined) PV reads k-tile k.
                plan = {u: [] for u in range(len(units))}

                def Q(a, b):
                    return lambda: emit_qk_block(a, b, 0)

                def Kb(a, b):
                    return lambda: emit_qk_block(a, b, 1)

                def Vb(rt):
                    return lambda: emit_v_block(rt)

                upfront = [Q(0, 0), Kb(0, 0),
                           Vb(0), Vb(1), Vb(2), Vb(3)]
                # unit 0: remaining K chunks of hp0 at groups 0/2/4 (needed
                # at 2c), V[4..15] at groups 1..6, Q(0,1) at 7
                for c in range(1, QC):
                    plan[0].append((2 * c - 2, Kb(0, c)))
                for k in range(4, KT):
                    plan[0].append(((k - 4) // 2 + 1, Vb(k)))
                plan[0].append((7, Q(0, 1)))
                plan[1] = [(2, Q(0, 2)), (5, Kb(1, 0))]
                plan[2] = [(2, Q(0, 3)), (5, Kb(1, 1))]
                plan[3] = [(2, Q(1, 0)), (4, Kb(1, 2)), (6, Kb(1, 3))]
                plan[4] = [(2, Q(1, 1))]
                plan[5] = [(2, Q(1, 2))]
                plan[6] = [(2, Q(1, 3))]

                for fn_ in upfront:
                    fn_()

                GS = 2
                NG = KT // GS
                for u, (hp, qc) in enumerate(units):
                    heads = (2 * hp, 2 * hp + 1)  # local head ids
                    qs_ = slice(qc * QW, (qc + 1) * QW)
                    mT = [mT_psum.tile([P, QW], F32, tag="mT", name="mT") for _ in range(2)]
                    pT_slices = [[], []]

                    def emit_pv(g):
                        for i, h in enumerate(heads):
                            for uu in range(GS):
                                kt = GS * g + uu
                                nc.tensor.matmul(
                                    mT[i][:],
                                    mmcast(Vt[kt][:, h * V:(h + 1) * V]),
                                    mmcast(pT_slices[i][kt]),
                                    start=(kt == 0), stop=(kt == KT - 1))

                    for g in range(NG):
                        for i, h in enumerate(heads):
                            po = (h % 2) * D  # partition offset in QT/KT tile
                            sT = sT_psum.tile([P, GS * QW], F32, tag="sT", name="sT")
                            for uu in range(GS):
                                kt = GS * g + uu
                                nc.tensor.matmul(
                                    sT[:, uu * QW:(uu + 1) * QW],
                                    mmcast(KTt[hp][po:po + D, kt * P:(kt + 1) * P]),
                                    mmcast(QT[hp][po:po + D, qs_]),
                                    start=True, stop=True)
                            pT = pT_pool.tile([P, GS * QW], mmdt, tag="pT", name="pT")
                            nc.scalar.activation(
                                pT[:], sT[:],
                                mybir.ActivationFunctionType.Exp, scale=SCALE)
                            pT_slices[i].extend(
                                pT[:, uu * QW:(uu + 1) * QW] for uu in range(GS))
                        for gg, blk in plan[u]:
                            if gg == g:
                                blk()
                        if g > 0:
                            emit_pv(g - 1)
                    emit_pv(NG - 1)

                    # column-tiled sums (4 concurrent M=32 matmuls on col
                    # groups 0/32/64/96; rows replicate the partial sums),
                    # then per-q collapse via a [128x1] (1/32) fp32 matmul.
                    mTs = [cwork.tile([P, QW], F32, tag="mTs", name="mTs") for _ in range(2)]
                    s4 = [cwork.tile([P, QW], F32, tag="s4", name="s4") for _ in range(2)]
                    for i in range(2):
                        sm = scr_psum.tile([P, QW], F32, tag="scr", name="sm")
                        for r in range(4):
                            for j in range(4):
                                nc.tensor.matmul(
                                    sm[32 * j:32 * (j + 1), :],
                                    mmcast(ones32[:]),
                                    mmcast(pT_slices[i][4 * r + j]),
                                    start=(r == 0), stop=(r == 3),
                                    tile_position=(0, 32 * j),
                                    skip_group_check=True)
                        nc.vector.tensor_copy(s4[i][:], sm[:])
                        nc.vector.tensor_copy(mTs[i][:], mT[i][:])
                    for qs in range(QW // P):
                        for i, h in enumerate(heads):
                            stp = scr_psum.tile([P, P], F32, tag="scr", name="stp")
                            nc.tensor.matmul(
                                stp[:, 0:1], s4[i][:, qs * P:(qs + 1) * P],
                                inv32[:], start=True, stop=True)
                            rcp = cwork.tile([P, 1], F32, tag="rcp")
                            nc.vector.reciprocal(rcp[:], stp[:, 0:1])
                            otp = scr_psum.tile([P, P], F32, tag="scr", name="otp")
                            nc.tensor.transpose(
                                otp[:], mTs[i][:, qs * P:(qs + 1) * P], identf[:])
                            ob = cwork.tile([P, P], F32, tag="ob")
                            nc.vector.tensor_scalar_mul(ob[:], otp[:], rcp[:])
                            nc.sync.dma_start(
                                out=out_d[qc * QW + qs * P:qc * QW + (qs + 1) * P,
                                          h * V:(h + 1) * V],
                                in_=ob[:])

    nc.compile()
    return nc


_CACHE = {}


def _get_nc(mode: str, repeat: int = 1):
    key = (mode, repeat)
    if key not in _CACHE:
        _CACHE[key] = build_nc(mode, repeat)
    return _CACHE[key]


def make_in_maps(x, Wq, bq, Wk, bk, Wv, bv):
    """Shard full inputs into 8 per-core input maps."""
    x = np.ascontiguousarray(np.asarray(x, dtype=np.float32))
    Wq = np.asarray(Wq, np.float32); bq = np.asarray(bq, np.float32)
    Wk = np.asarray(Wk, np.float32); bk = np.asarray(bk, np.float32)
    Wv = np.asarray(Wv, np.float32); bv = np.asarray(bv, np.float32)
    in_maps = []
    for c in range(8):
        b, g = c // 2, c % 2
        qsl = slice(g * HC * D, (g + 1) * HC * D)
        vsl = slice(g * HC * V, (g + 1) * HC * V)
        in_maps.append({
            "x": np.ascontiguousarray(x[b]),
            "wq": np.ascontiguousarray(Wq[:, qsl]),
            "bq": np.ascontiguousarray(bq[qsl]),
            "wk": np.ascontiguousarray(Wk[:, qsl]),
            "bk": np.ascontiguousarray(bk[qsl]),
            "wv": np.ascontiguousarray(Wv[:, vsl]),
            "bv": np.ascontiguousarray(bv[vsl]),
        })
    return in_maps


def gather_out(results):
    full = np.empty((B, N, 1024), np.float32)
    for c in range(8):
        b, g = c // 2, c % 2
        full[b, :, g * HC * V:(g + 1) * HC * V] = results[c]["out"]
    return full


def kernel(x, Wq, bq, Wk, bk, Wv, bv):
    nc = _get_nc("bf16")
    in_maps = make_in_maps(x, Wq, bq, Wk, bk, Wv, bv)
    res = run_bass_kernel_spmd(nc, in_maps, list(range(8)))
    return gather_out(res.results)


# revision 24
# speedup vs baseline: 1.6531x; 1.6531x over previous
"""Multi-head self-attention Trainium2 Bass kernel.

Full-input contract: kernel(**inputs) takes the unsharded inputs
(x [4,2048,1024], Wq [1024,512], bq [512], Wk, bk, Wv [1024,1024], bv)
and returns the full [4,2048,1024] output.

Sharding: 8 cores = 4 batches x 2 head-groups. Core c handles batch c//2
and heads 4*(c%2) .. 4*(c%2)+4. Pure SPMD, no collectives.

Per-core algorithm (N=2048 rows, C=1024, 4 heads, d=64, v=128):
  - load x naturally, PE-transpose into xT (C on partitions)
  - QT/KT = W.T @ xT with head-dim on partitions; V natural (rows on
    partitions); biases folded in via rank-1 (K=1) matmuls
  - scores computed TRANSPOSED: sT[keys,q] = (KT tile).T @ QT, so that
    exp(sT) (ACT, scale fused) is directly the PV rhs operand. No max
    subtraction (|scale*s| < ~4 -> exp safely in fp32 range).
  - msgT[v,q] accumulates over key tiles; row-sums of exp via ones-lhsT
    matmuls (column-tiled 4-wide so 4 run concurrently on the PE array);
    per-q normalization applied after transposing back to natural layout.
"""

import math
import os

import numpy as np

import concourse.bass as bass
import concourse.mybir as mybir
import concourse.tile as tile
from concourse import bacc
from concourse.bass_utils import run_bass_kernel_spmd
from concourse.masks import make_identity

F32 = mybir.dt.float32
BF16 = mybir.dt.bfloat16
F32R = mybir.dt.float32r

# dims
B, N, C = 4, 2048, 1024
QK_DIM, NHEADS = 512, 8
D = QK_DIM // NHEADS          # 64 per-head qk dim
V = 1024 // NHEADS            # 128 per-head value dim
SCALE = 1.0 / math.sqrt(D)
HC = 4                        # heads per core
P = 128
NT = N // P                   # 16 row tiles
CT = C // P                   # 8 contraction tiles
KT = N // P                   # 16 key tiles
QC = 4                        # q chunks of 512
QW = N // QC                  # 512


def build_nc(mode: str = "bf16", repeat: int = 1):
    """Build the per-core Bass program (bf16 matmul operands, fp32 psum)."""
    mmdt = BF16
    mmcast = lambda ap: ap

    nc = bacc.Bacc("TRN2", target_bir_lowering=False, debug=False, num_devices=8)

    x_d = nc.dram_tensor("x", [N, C], F32, kind="ExternalInput").ap()
    wq_d = nc.dram_tensor("wq", [C, HC * D], F32, kind="ExternalInput").ap()
    bq_d = nc.dram_tensor("bq", [HC * D], F32, kind="ExternalInput").ap()
    wk_d = nc.dram_tensor("wk", [C, HC * D], F32, kind="ExternalInput").ap()
    bk_d = nc.dram_tensor("bk", [HC * D], F32, kind="ExternalInput").ap()
    wv_d = nc.dram_tensor("wv", [C, HC * V], F32, kind="ExternalInput").ap()
    bv_d = nc.dram_tensor("bv", [HC * V], F32, kind="ExternalInput").ap()
    out_d = nc.dram_tensor("out", [N, HC * V], F32, kind="ExternalOutput").ap()

    with tile.TileContext(nc) as tc:
      for _rep in range(repeat):
        with tc.tile_pool(name="persist", bufs=1) as persist:
            # persistent SBUF arrays
            xT = [persist.tile([P, N], mmdt, tag=f"xT{ct}", name=f"xT{ct}") for ct in range(CT)]
            QT = [persist.tile([P, N], mmdt, tag=f"QT{hp}", name=f"QT{hp}") for hp in range(2)]
            KTt = [persist.tile([P, N], mmdt, tag=f"KT{hp}", name=f"KT{hp}") for hp in range(2)]
            Vt = [persist.tile([P, HC * V], mmdt, tag=f"V{rt}", name=f"V{rt}") for rt in range(NT)]

            ident = persist.tile([P, P], mmdt, tag="ident")
            make_identity(nc, ident)
            identf = persist.tile([P, P], F32, tag="identf")
            make_identity(nc, identf)

            ones_row = persist.tile([1, QW], mmdt, tag="ones_row")
            nc.vector.memset(ones_row[:], 1.0)
            ones32 = persist.tile([P, 32], mmdt, tag="ones32")
            nc.vector.memset(ones32[:], 1.0)
            inv32 = persist.tile([P, 1], F32, tag="inv32")
            nc.vector.memset(inv32[:], 1.0 / 32.0)

            # biases (as [1, n] rows in matmul dtype)
            bq_sb = persist.tile([1, HC * D], mmdt, tag="bq")
            bk_sb = persist.tile([1, HC * D], mmdt, tag="bk")
            bv_sb = persist.tile([1, HC * V], mmdt, tag="bv")
            # weights in matmul dtype
            wv_sb = [persist.tile([P, HC * V], mmdt, tag=f"wv{ct}", name=f"wv{ct}") for ct in range(CT)]
            wq_sb = [persist.tile([P, HC * D], mmdt, tag=f"wqf{ct}", name=f"wqf{ct}") for ct in range(CT)]
            wk_sb = [persist.tile([P, HC * D], mmdt, tag=f"wkf{ct}", name=f"wkf{ct}") for ct in range(CT)]

            # ---- Phase A: load x (critical path: issue x DMAs first),
            #      transpose into xT ----
            with tc.tile_pool(name="xload", bufs=3) as xload, \
                 tc.tile_pool(name="stage", bufs=3) as stage, \
                 tc.tile_pool(name="tp_psum", bufs=3, space="PSUM") as tp_psum, \
                 tc.tile_pool(name="pp_psum", bufs=3, space="PSUM") as pp_psum:
                xns = []
                for rt in range(NT):
                    xn = xload.tile([P, C], F32, tag="xn", bufs=6, name="xn")
                    eng = nc.sync if rt % 2 == 0 else nc.scalar
                    eng.dma_start(out=xn[:], in_=x_d[rt * P:(rt + 1) * P, :])
                    xns.append(xn)

                # weight/bias staging on a different DMA engine so it does
                # not delay the x loads
                for bd, bs, n_ in ((bq_d, bq_sb, HC * D), (bk_d, bk_sb, HC * D),
                                   (bv_d, bv_sb, HC * V)):
                    st = stage.tile([1, n_], F32, tag="bias_st", name="b_st")
                    nc.scalar.dma_start(out=st[:], in_=bd.unsqueeze(0))
                    nc.vector.tensor_copy(bs[:], st[:])
                for ct in range(CT):
                    for wd, ws, n_ in ((wq_d, wq_sb[ct], HC * D),
                                       (wk_d, wk_sb[ct], HC * D),
                                       (wv_d, wv_sb[ct], HC * V)):
                        st = stage.tile([P, n_], F32, tag="w_st", name="w_st")
                        nc.scalar.dma_start(out=st[:], in_=wd[ct * P:(ct + 1) * P, :])
                        nc.vector.tensor_copy(ws[:], st[:])

                for rt in range(NT):
                    xn = xns[rt]
                    xb = xload.tile([P, C], mmdt, tag="xb", name="xb")
                    nc.vector.tensor_copy(xb[:], xn[:])
                    for ct in range(CT):
                        pt = tp_psum.tile([P, P], mmdt, name="pt")
                        nc.tensor.transpose(pt[:], xb[:, ct * P:(ct + 1) * P], ident[:])
                        # alternate eviction engine to split the load
                        if ct % 2 == 0:
                            nc.vector.tensor_copy(xT[ct][:, rt * P:(rt + 1) * P], pt[:])
                        else:
                            nc.scalar.copy(xT[ct][:, rt * P:(rt + 1) * P], pt[:])

                # ---- Phases B+C interleaved ----
            # Phase C is ACT(exp)-bound while projections are PE-only, so
            # projection blocks are emitted INTO the attention loop where the
            # PE would otherwise idle. One 2-slot PSUM scratch ring serves
            # proj blocks, sums banks and output transposes (their lifetimes
            # never overlap).
            n_pt_bufs = KT + 2
            with tc.tile_pool(name="sT_psum", bufs=2, space="PSUM") as sT_psum, \
                 tc.tile_pool(name="mT_psum", bufs=2, space="PSUM") as mT_psum, \
                 tc.tile_pool(name="scr_psum", bufs=2, space="PSUM") as scr_psum, \
                 tc.tile_pool(name="pT_pool", bufs=n_pt_bufs) as pT_pool, \
                 tc.tile_pool(name="cwork", bufs=2) as cwork:

                def emit_qk_block(hp, qc, which):
                    w_sb, b_sb, dst = ((wq_sb, bq_sb, QT) if which == 0
                                       else (wk_sb, bk_sb, KTt))
                    ps = scr_psum.tile([P, QW], F32, tag="scr", name="ppqk")
                    for ct in range(CT):
                        nc.tensor.matmul(
                            ps[:],
                            mmcast(w_sb[ct][:, hp * P:(hp + 1) * P]),
                            mmcast(xT[ct][:, qc * QW:(qc + 1) * QW]),
                            start=(ct == 0), stop=False)
                    nc.tensor.matmul(
                        ps[:], mmcast(b_sb[:, hp * P:(hp + 1) * P]),
                        mmcast(ones_row[:]), start=False, stop=True)
                    nc.vector.tensor_copy(dst[hp][:, qc * QW:(qc + 1) * QW], ps[:])

                def emit_v_block(rt):
                    ps = scr_psum.tile([P, HC * V], F32, tag="scr", name="ppv")
                    for ct in range(CT):
                        nc.tensor.matmul(
                            ps[:],
                            mmcast(xT[ct][:, rt * P:(rt + 1) * P]),
                            mmcast(wv_sb[ct][:]),
                            start=(ct == 0), stop=False)
                    nc.tensor.matmul(
                        ps[:], mmcast(ones_row[:, 0:P]),
                        mmcast(bv_sb[:]), start=False, stop=True)
                    nc.vector.tensor_copy(Vt[rt][:], ps[:])

                units = [(hp, qc) for hp in range(2) for qc in range(QC)]
                # pending projection work, consumed during attention loops.
                # Deps: unit (hp,qc) needs Q(hp,qc) at start, K(hp,c) by
                # group 2c (its sT scans ALL key chunks), V[k] by the group
                # whose (pipelined) PV reads k-tile k.
                plan = {u: [] for u in range(len(units))}

                def Q(a, b):
                    return lambda: emit_qk_block(a, b, 0)

                def Kb(a, b):
                    return lambda: emit_qk_block(a, b, 1)

                def Vb(rt):
                    return lambda: emit_v_block(rt)

                upfront = [Q(0, 0), Kb(0, 0),
                           Vb(0), Vb(1), Vb(2), Vb(3)]
                # unit 0: remaining K chunks of hp0 at groups 0/2/4 (needed
                # at 2c), V[4..15] at groups 1..6, Q(0,1) at 7
                for c in range(1, QC):
                    plan[0].append((2 * c - 2, Kb(0, c)))
                for k in range(4, KT):
                    plan[0].append(((k - 4) // 2 + 1, Vb(k)))
                plan[0].append((7, Q(0, 1)))
                plan[1] = [(2, Q(0, 2)), (5, Kb(1, 0))]
                plan[2] = [(2, Q(0, 3)), (5, Kb(1, 1))]
                plan[3] = [(2, Q(1, 0)), (4, Kb(1, 2)), (6, Kb(1, 3))]
                plan[4] = [(2, Q(1, 1))]
                plan[5] = [(2, Q(1, 2))]
                plan[6] = [(2, Q(1, 3))]

                for fn_ in upfront:
                    fn_()

                GS = 2
                NG = KT // GS
                for u, (hp, qc) in enumerate(units):
                    heads = (2 * hp, 2 * hp + 1)  # local head ids
                    qs_ = slice(qc * QW, (qc + 1) * QW)
                    mT = [mT_psum.tile([P, QW], F32, tag="mT", name="mT") for _ in range(2)]
                    pT_slices = [[], []]

                    def emit_pv(g):
                        for i, h in enumerate(heads):
                            for uu in range(GS):
                                kt = GS * g + uu
                                nc.tensor.matmul(
                                    mT[i][:],
                                    mmcast(Vt[kt][:, h * V:(h + 1) * V]),
                                    mmcast(pT_slices[i][kt]),
                                    start=(kt == 0), stop=(kt == KT - 1))

                    for g in range(NG):
                        for i, h in enumerate(heads):
                            po = (h % 2) * D  # partition offset in QT/KT tile
                            sT = sT_psum.tile([P, GS * QW], F32, tag="sT", name="sT")
                            for uu in range(GS):
                                kt = GS * g + uu
                                nc.tensor.matmul(
                                    sT[:, uu * QW:(uu + 1) * QW],
                                    mmcast(KTt[hp][po:po + D, kt * P:(kt + 1) * P]),
                                    mmcast(QT[hp][po:po + D, qs_]),
                                    start=True, stop=True)
                            pT = pT_pool.tile([P, GS * QW], mmdt, tag="pT", name="pT")
                            nc.scalar.activation(
                                pT[:], sT[:],
                                mybir.ActivationFunctionType.Exp, scale=SCALE)
                            pT_slices[i].extend(
                                pT[:, uu * QW:(uu + 1) * QW] for uu in range(GS))
                        for gg, blk in plan[u]:
                            if gg == g:
                                blk()
                        if g > 0:
                            emit_pv(g - 1)
                    emit_pv(NG - 1)

                    # column-tiled sums (4 concurrent M=32 matmuls on col
                    # groups 0/32/64/96; rows replicate the partial sums),
                    # then per-q collapse via a [128x1] (1/32) fp32 matmul.
                    mTs = [cwork.tile([P, QW], F32, tag="mTs", name="mTs") for _ in range(2)]
                    s4 = [cwork.tile([P, QW], F32, tag="s4", name="s4") for _ in range(2)]
                    for i in range(2):
                        sm = scr_psum.tile([P, QW], F32, tag="scr", name="sm")
                        for r in range(4):
                            for j in range(4):
                                nc.tensor.matmul(
                                    sm[32 * j:32 * (j + 1), :],
                                    mmcast(ones32[:]),
                                    mmcast(pT_slices[i][4 * r + j]),
                                    start=(r == 0), stop=(r == 3),
                                    tile_position=(0, 32 * j),
                                    skip_group_check=True)
                        nc.vector.tensor_copy(s4[i][:], sm[:])
                        nc.vector.tensor_copy(mTs[i][:], mT[i][:])
                    for qs in range(QW // P):
                        for i, h in enumerate(heads):
                            stp = scr_psum.tile([P, P], F32, tag="scr", name="stp")
                            nc.tensor.matmul(
                                stp[:, 0:1], s4[i][:, qs * P:(qs + 1) * P],
                                inv32[:], start=True, stop=True)
                            rcp = cwork.tile([P, 1], F32, tag="rcp")
                            nc.vector.reciprocal(rcp[:], stp[:, 0:1])
                            otp = scr_psum.tile([P, P], F32, tag="scr", name="otp")
                            nc.tensor.transpose(
                                otp[:], mTs[i][:, qs * P:(qs + 1) * P], identf[:])
                            ob = cwork.tile([P, P], F32, tag="ob")
                            nc.vector.tensor_scalar_mul(ob[:], otp[:], rcp[:])
                            nc.sync.dma_start(
                                out=out_d[qc * QW + qs * P:qc * QW + (qs + 1) * P,
                                          h * V:(h + 1) * V],
                                in_=ob[:])

    nc.compile()
    return nc


_CACHE = {}


def _get_nc(mode: str, repeat: int = 1):
    key = (mode, repeat)
    if key not in _CACHE:
        _CACHE[key] = build_nc(mode, repeat)
    return _CACHE[key]


def make_in_maps(x, Wq, bq, Wk, bk, Wv, bv):
    """Shard full inputs into 8 per-core input maps."""
    x = np.ascontiguousarray(np.asarray(x, dtype=np.float32))
    Wq = np.asarray(Wq, np.float32); bq = np.asarray(bq, np.float32)
    Wk = np.asarray(Wk, np.float32); bk = np.asarray(bk, np.float32)
    Wv = np.asarray(Wv, np.float32); bv = np.asarray(bv, np.float32)
    in_maps = []
    for c in range(8):
        b, g = c // 2, c % 2
        qsl = slice(g * HC * D, (g + 1) * HC * D)
        vsl = slice(g * HC * V, (g + 1) * HC * V)
        in_maps.append({
            "x": np.ascontiguousarray(x[b]),
            "wq": np.ascontiguousarray(Wq[:, qsl]),
            "bq": np.ascontiguousarray(bq[qsl]),
            "wk": np.ascontiguousarray(Wk[:, qsl]),
            "bk": np.ascontiguousarray(bk[qsl]),
            "wv": np.ascontiguousarray(Wv[:, vsl]),
            "bv": np.ascontiguousarray(bv[vsl]),
        })
    return in_maps


def gather_out(results):
    full = np.empty((B, N, 1024), np.float32)
    for c in range(8):
        b, g = c // 2, c % 2
        full[b, :, g * HC * V:(g + 1) * HC * V] = results[c]["out"]
    return full


def kernel(x, Wq, bq, Wk, bk, Wv, bv):
    nc = _get_nc("bf16")
    in_maps = make_in_maps(x, Wq, bq, Wk, bk, Wv, bv)
    res = run_bass_kernel_spmd(nc, in_maps, list(range(8)))
    return gather_out(res.results)


# revision 25
# speedup vs baseline: 1.8294x; 1.1066x over previous
"""Multi-head self-attention Trainium2 Bass kernel.

Full-input contract: kernel(**inputs) takes the unsharded inputs
(x [4,2048,1024], Wq [1024,512], bq [512], Wk, bk, Wv [1024,1024], bv)
and returns the full [4,2048,1024] output.

Sharding: 8 cores = 4 batches x 2 head-groups. Core c handles batch c//2
and heads 4*(c%2) .. 4*(c%2)+4. Pure SPMD, no collectives.

Per-core algorithm (N=2048 rows, C=1024, 4 heads, d=64, v=128):
  - load x naturally, PE-transpose into xT (C on partitions)
  - QT/KT = W.T @ xT with head-dim on partitions; V natural (rows on
    partitions); biases folded in via rank-1 (K=1) matmuls
  - scores computed TRANSPOSED: sT[keys,q] = (KT tile).T @ QT, so that
    exp(sT) (ACT, scale fused) is directly the PV rhs operand. No max
    subtraction (|scale*s| < ~4 -> exp safely in fp32 range).
  - msgT[v,q] accumulates over key tiles; row-sums of exp via ones-lhsT
    matmuls (column-tiled 4-wide so 4 run concurrently on the PE array);
    per-q normalization applied after transposing back to natural layout.
"""

import math
import os

import numpy as np

import concourse.bass as bass
import concourse.mybir as mybir
import concourse.tile as tile
from concourse import bacc
from concourse.bass_utils import run_bass_kernel_spmd
from concourse.masks import make_identity

F32 = mybir.dt.float32
BF16 = mybir.dt.bfloat16
F32R = mybir.dt.float32r

# dims
B, N, C = 4, 2048, 1024
QK_DIM, NHEADS = 512, 8
D = QK_DIM // NHEADS          # 64 per-head qk dim
V = 1024 // NHEADS            # 128 per-head value dim
SCALE = 1.0 / math.sqrt(D)
HC = 4                        # heads per core
P = 128
NT = N // P                   # 16 row tiles
CT = C // P                   # 8 contraction tiles
KT = N // P                   # 16 key tiles
QC = 4                        # q chunks of 512
QW = N // QC                  # 512


def build_nc(mode: str = "bf16", repeat: int = 1):
    """Build the per-core Bass program (bf16 matmul operands, fp32 psum)."""
    mmdt = BF16
    mmcast = lambda ap: ap

    nc = bacc.Bacc("TRN2", target_bir_lowering=False, debug=False, num_devices=8)

    x_d = nc.dram_tensor("x", [N, C], F32, kind="ExternalInput").ap()
    wq_d = nc.dram_tensor("wq", [C, HC * D], F32, kind="ExternalInput").ap()
    bq_d = nc.dram_tensor("bq", [HC * D], F32, kind="ExternalInput").ap()
    wk_d = nc.dram_tensor("wk", [C, HC * D], F32, kind="ExternalInput").ap()
    bk_d = nc.dram_tensor("bk", [HC * D], F32, kind="ExternalInput").ap()
    wv_d = nc.dram_tensor("wv", [C, HC * V], F32, kind="ExternalInput").ap()
    bv_d = nc.dram_tensor("bv", [HC * V], F32, kind="ExternalInput").ap()
    out_d = nc.dram_tensor("out", [N, HC * V], F32, kind="ExternalOutput").ap()

    with tile.TileContext(nc) as tc:
      for _rep in range(repeat):
        with tc.tile_pool(name="persist", bufs=1) as persist:
            # persistent SBUF arrays
            xT = [persist.tile([P, N], mmdt, tag=f"xT{ct}", name=f"xT{ct}") for ct in range(CT)]
            QT = [persist.tile([P, N], mmdt, tag=f"QT{hp}", name=f"QT{hp}") for hp in range(2)]
            KTt = [persist.tile([P, N], mmdt, tag=f"KT{hp}", name=f"KT{hp}") for hp in range(2)]
            Vt = [persist.tile([P, HC * V], mmdt, tag=f"V{rt}", name=f"V{rt}") for rt in range(NT)]

            ident = persist.tile([P, P], mmdt, tag="ident")
            make_identity(nc, ident)
            identf = persist.tile([P, P], F32, tag="identf")
            make_identity(nc, identf)

            ones_row = persist.tile([1, QW], mmdt, tag="ones_row")
            nc.vector.memset(ones_row[:], 1.0)
            ones32 = persist.tile([P, 32], mmdt, tag="ones32")
            nc.vector.memset(ones32[:], 1.0)
            inv32 = persist.tile([P, 1], F32, tag="inv32")
            nc.vector.memset(inv32[:], 1.0 / 32.0)

            # biases (as [1, n] rows in matmul dtype)
            bq_sb = persist.tile([1, HC * D], mmdt, tag="bq")
            bk_sb = persist.tile([1, HC * D], mmdt, tag="bk")
            bv_sb = persist.tile([1, HC * V], mmdt, tag="bv")
            # weights in matmul dtype
            wv_sb = [persist.tile([P, HC * V], mmdt, tag=f"wv{ct}", name=f"wv{ct}") for ct in range(CT)]
            wq_sb = [persist.tile([P, HC * D], mmdt, tag=f"wqf{ct}", name=f"wqf{ct}") for ct in range(CT)]
            wk_sb = [persist.tile([P, HC * D], mmdt, tag=f"wkf{ct}", name=f"wkf{ct}") for ct in range(CT)]

            # ---- Phase A: load x (critical path: issue x DMAs first),
            #      transpose into xT ----
            with tc.tile_pool(name="xload", bufs=3) as xload, \
                 tc.tile_pool(name="stage", bufs=3) as stage, \
                 tc.tile_pool(name="tp_psum", bufs=3, space="PSUM") as tp_psum, \
                 tc.tile_pool(name="pp_psum", bufs=3, space="PSUM") as pp_psum:
                xns = []
                for rt in range(NT):
                    xn = xload.tile([P, C], F32, tag="xn", bufs=6, name="xn")
                    eng = nc.sync if rt % 2 == 0 else nc.scalar
                    eng.dma_start(out=xn[:], in_=x_d[rt * P:(rt + 1) * P, :])
                    xns.append(xn)

                # weight/bias staging on a different DMA engine so it does
                # not delay the x loads
                for bd, bs, n_ in ((bq_d, bq_sb, HC * D), (bk_d, bk_sb, HC * D),
                                   (bv_d, bv_sb, HC * V)):
                    st = stage.tile([1, n_], F32, tag="bias_st", name="b_st")
                    nc.gpsimd.dma_start(out=st[:], in_=bd.unsqueeze(0))
                    nc.vector.tensor_copy(bs[:], st[:])
                for ct in range(CT):
                    for wd, ws, n_ in ((wq_d, wq_sb[ct], HC * D),
                                       (wk_d, wk_sb[ct], HC * D),
                                       (wv_d, wv_sb[ct], HC * V)):
                        st = stage.tile([P, n_], F32, tag="w_st", name="w_st")
                        nc.gpsimd.dma_start(out=st[:], in_=wd[ct * P:(ct + 1) * P, :])
                        nc.vector.tensor_copy(ws[:], st[:])

                for rt in range(NT):
                    xn = xns[rt]
                    for ct in range(CT):
                        pt = tp_psum.tile([P, P], F32, name="pt")
                        nc.tensor.transpose(pt[:], xn[:, ct * P:(ct + 1) * P], identf[:])
                        # alternate eviction engine to split the load
                        if ct % 2 == 0:
                            nc.vector.tensor_copy(xT[ct][:, rt * P:(rt + 1) * P], pt[:])
                        else:
                            nc.scalar.copy(xT[ct][:, rt * P:(rt + 1) * P], pt[:])

                # ---- Phases B+C interleaved ----
            # Phase C is ACT(exp)-bound while projections are PE-only, so
            # projection blocks are emitted INTO the attention loop where the
            # PE would otherwise idle. One 2-slot PSUM scratch ring serves
            # proj blocks, sums banks and output transposes (their lifetimes
            # never overlap).
            n_pt_bufs = KT + 2
            with tc.tile_pool(name="sT_psum", bufs=2, space="PSUM") as sT_psum, \
                 tc.tile_pool(name="mT_psum", bufs=2, space="PSUM") as mT_psum, \
                 tc.tile_pool(name="scr_psum", bufs=2, space="PSUM") as scr_psum, \
                 tc.tile_pool(name="pT_pool", bufs=n_pt_bufs) as pT_pool, \
                 tc.tile_pool(name="cwork", bufs=2) as cwork:

                def emit_qk_block(hp, qc, which):
                    w_sb, b_sb, dst = ((wq_sb, bq_sb, QT) if which == 0
                                       else (wk_sb, bk_sb, KTt))
                    ps = scr_psum.tile([P, QW], F32, tag="scr", name="ppqk")
                    for ct in range(CT):
                        nc.tensor.matmul(
                            ps[:],
                            mmcast(w_sb[ct][:, hp * P:(hp + 1) * P]),
                            mmcast(xT[ct][:, qc * QW:(qc + 1) * QW]),
                            start=(ct == 0), stop=False)
                    nc.tensor.matmul(
                        ps[:], mmcast(b_sb[:, hp * P:(hp + 1) * P]),
                        mmcast(ones_row[:]), start=False, stop=True)
                    nc.vector.tensor_copy(dst[hp][:, qc * QW:(qc + 1) * QW], ps[:])

                def emit_v_block(rt):
                    ps = scr_psum.tile([P, HC * V], F32, tag="scr", name="ppv")
                    for ct in range(CT):
                        nc.tensor.matmul(
                            ps[:],
                            mmcast(xT[ct][:, rt * P:(rt + 1) * P]),
                            mmcast(wv_sb[ct][:]),
                            start=(ct == 0), stop=False)
                    nc.tensor.matmul(
                        ps[:], mmcast(ones_row[:, 0:P]),
                        mmcast(bv_sb[:]), start=False, stop=True)
                    nc.vector.tensor_copy(Vt[rt][:], ps[:])

                units = [(hp, qc) for hp in range(2) for qc in range(QC)]
                # pending projection work, consumed during attention loops.
                # Deps: unit (hp,qc) needs Q(hp,qc) at start, K(hp,c) by
                # group 2c (its sT scans ALL key chunks), V[k] by the group
                # whose (pipelined) PV reads k-tile k.
                plan = {u: [] for u in range(len(units))}

                def Q(a, b):
                    return lambda: emit_qk_block(a, b, 0)

                def Kb(a, b):
                    return lambda: emit_qk_block(a, b, 1)

                def Vb(rt):
                    return lambda: emit_v_block(rt)

                upfront = [Q(0, 0), Kb(0, 0),
                           Vb(0), Vb(1), Vb(2), Vb(3)]
                # unit 0: remaining K chunks of hp0 at groups 0/2/4 (needed
                # at 2c), V[4..15] at groups 1..6, Q(0,1) at 7
                for c in range(1, QC):
                    plan[0].append((2 * c - 2, Kb(0, c)))
                for k in range(4, KT):
                    plan[0].append(((k - 4) // 2 + 1, Vb(k)))
                plan[0].append((7, Q(0, 1)))
                plan[1] = [(2, Q(0, 2)), (5, Kb(1, 0))]
                plan[2] = [(2, Q(0, 3)), (5, Kb(1, 1))]
                plan[3] = [(2, Q(1, 0)), (4, Kb(1, 2)), (6, Kb(1, 3))]
                plan[4] = [(2, Q(1, 1))]
                plan[5] = [(2, Q(1, 2))]
                plan[6] = [(2, Q(1, 3))]

                for fn_ in upfront:
                    fn_()

                GS = 2
                NG = KT // GS
                for u, (hp, qc) in enumerate(units):
                    heads = (2 * hp, 2 * hp + 1)  # local head ids
                    qs_ = slice(qc * QW, (qc + 1) * QW)
                    mT = [mT_psum.tile([P, QW], F32, tag="mT", name="mT") for _ in range(2)]
                    pT_slices = [[], []]

                    def emit_pv(g):
                        for i, h in enumerate(heads):
                            for uu in range(GS):
                                kt = GS * g + uu
                                nc.tensor.matmul(
                                    mT[i][:],
                                    mmcast(Vt[kt][:, h * V:(h + 1) * V]),
                                    mmcast(pT_slices[i][kt]),
                                    start=(kt == 0), stop=(kt == KT - 1))

                    for g in range(NG):
                        for i, h in enumerate(heads):
                            po = (h % 2) * D  # partition offset in QT/KT tile
                            sT = sT_psum.tile([P, GS * QW], F32, tag="sT", name="sT")
                            for uu in range(GS):
                                kt = GS * g + uu
                                nc.tensor.matmul(
                                    sT[:, uu * QW:(uu + 1) * QW],
                                    mmcast(KTt[hp][po:po + D, kt * P:(kt + 1) * P]),
                                    mmcast(QT[hp][po:po + D, qs_]),
                                    start=True, stop=True)
                            pT = pT_pool.tile([P, GS * QW], mmdt, tag="pT", name="pT")
                            nc.scalar.activation(
                                pT[:], sT[:],
                                mybir.ActivationFunctionType.Exp, scale=SCALE)
                            pT_slices[i].extend(
                                pT[:, uu * QW:(uu + 1) * QW] for uu in range(GS))
                        for gg, blk in plan[u]:
                            if gg == g:
                                blk()
                        if g > 0:
                            emit_pv(g - 1)
                    emit_pv(NG - 1)

                    # column-tiled sums (4 concurrent M=32 matmuls on col
                    # groups 0/32/64/96; rows replicate the partial sums),
                    # then per-q collapse via a [128x1] (1/32) fp32 matmul.
                    mTs = [cwork.tile([P, QW], F32, tag="mTs", name="mTs") for _ in range(2)]
                    s4 = [cwork.tile([P, QW], F32, tag="s4", name="s4") for _ in range(2)]
                    for i in range(2):
                        sm = scr_psum.tile([P, QW], F32, tag="scr", name="sm")
                        for r in range(4):
                            for j in range(4):
                                nc.tensor.matmul(
                                    sm[32 * j:32 * (j + 1), :],
                                    mmcast(ones32[:]),
                                    mmcast(pT_slices[i][4 * r + j]),
                                    start=(r == 0), stop=(r == 3),
                                    tile_position=(0, 32 * j),
                                    skip_group_check=True)
                        nc.vector.tensor_copy(s4[i][:], sm[:])
                        nc.vector.tensor_copy(mTs[i][:], mT[i][:])
                    for qs in range(QW // P):
                        for i, h in enumerate(heads):
                            stp = scr_psum.tile([P, P], F32, tag="scr", name="stp")
                            nc.tensor.matmul(
                                stp[:, 0:1], s4[i][:, qs * P:(qs + 1) * P],
                                inv32[:], start=True, stop=True)
                            rcp = cwork.tile([P, 1], F32, tag="rcp")
                            nc.vector.reciprocal(rcp[:], stp[:, 0:1])
                            otp = scr_psum.tile([P, P], F32, tag="scr", name="otp")
                            nc.tensor.transpose(
                                otp[:], mTs[i][:, qs * P:(qs + 1) * P], identf[:])
                            ob = cwork.tile([P, P], F32, tag="ob")
                            nc.vector.tensor_scalar_mul(ob[:], otp[:], rcp[:])
                            nc.sync.dma_start(
                                out=out_d[qc * QW + qs * P:qc * QW + (qs + 1) * P,
                                          h * V:(h + 1) * V],
                                in_=ob[:])

    nc.compile()
    return nc


_CACHE = {}


def _get_nc(mode: str, repeat: int = 1):
    key = (mode, repeat)
    if key not in _CACHE:
        _CACHE[key] = build_nc(mode, repeat)
    return _CACHE[key]


def make_in_maps(x, Wq, bq, Wk, bk, Wv, bv):
    """Shard full inputs into 8 per-core input maps."""
    x = np.ascontiguousarray(np.asarray(x, dtype=np.float32))
    Wq = np.asarray(Wq, np.float32); bq = np.asarray(bq, np.float32)
    Wk = np.asarray(Wk, np.float32); bk = np.asarray(bk, np.float32)
    Wv = np.asarray(Wv, np.float32); bv = np.asarray(bv, np.float32)
    in_maps = []
    for c in range(8):
        b, g = c // 2, c % 2
        qsl = slice(g * HC * D, (g + 1) * HC * D)
        vsl = slice(g * HC * V, (g + 1) * HC * V)
        in_maps.append({
            "x": np.ascontiguousarray(x[b]),
            "wq": np.ascontiguousarray(Wq[:, qsl]),
            "bq": np.ascontiguousarray(bq[qsl]),
            "wk": np.ascontiguousarray(Wk[:, qsl]),
            "bk": np.ascontiguousarray(bk[qsl]),
            "wv": np.ascontiguousarray(Wv[:, vsl]),
            "bv": np.ascontiguousarray(bv[vsl]),
        })
    return in_maps


def gather_out(results):
    full = np.empty((B, N, 1024), np.float32)
    for c in range(8):
        b, g = c // 2, c % 2
        full[b, :, g * HC * V:(g + 1) * HC * V] = results[c]["out"]
    return full


def kernel(x, Wq, bq, Wk, bk, Wv, bv):
    nc = _get_nc("bf16")
    in_maps = make_in_maps(x, Wq, bq, Wk, bk, Wv, bv)
    res = run_bass_kernel_spmd(nc, in_maps, list(range(8)))
    return gather_out(res.results)
